# revision 1
# baseline (speedup 1.0000x reference)
"""MoE layer (N=16384, D=1024, E=8, H=2048, top-2) on 8 trn2 NeuronCores.

Strategy: expert parallelism. The reference computes every expert densely but
only the top-2 survive the gather — so we dispatch each token to its two
routed experts only (4x compute saving). Core c owns expert c's weights; the
host computes the gating (bit-identically to the reference, CPU jax) and
all-to-all-dispatches gathered token batches; each core runs a dense
  y = (gelu(x @ W1 + b1) @ W2 + b2) * p
MLP over its batch with float32r matmuls (full PE rate, ~1e-4 precision);
the host scatter-adds the two expert contributions plus the residual.

Self-contained: only numpy/jax/concourse imports.
"""
import numpy as np

import concourse.bass as bass
import concourse.mybir as mybir
import concourse.tile as tile
from concourse.bass_utils import run_bass_kernel_spmd

N, D, E, H, TOP_K = 16384, 1024, 8, 2048, 2
P = 128
BGRAIN = 256     # capacity granularity; also the min/tail block width
BMAIN = 512      # main token block (moving dim per matmul)
KD = D // P      # 8 k-tiles over D
JH = H // P      # 16 h-tiles over H

TRACE = False          # test harness may flip this
TRACE_CORES = None     # e.g. list(range(8)) to profile every core
LAST_RESULTS = None    # BassKernelResults of the last device run

F32 = mybir.dt.float32
F32R = mybir.dt.float32r


def _split_excess_waits(nc, max_waits=1):
    """This walrus build rejects >1 sem-wait per instruction; Tile emits more.
    Move excess waits onto same-engine NOPs inserted right before."""
    for fn in nc.m.functions:
        for blk in fn.blocks:
            insts = list(blk.instructions)
            out = []
            changed = False
            for inst in insts:
                si = getattr(inst, "sync_info", None)
                if si is not None and si.on_wait and len(si.on_wait) > max_waits:
                    waits = list(si.on_wait)
                    excess, keep = waits[:-max_waits], waits[-max_waits:]
                    for i in range(0, len(excess), max_waits):
                        out.append(
                            mybir.InstNoOp(
                                name=nc.get_next_instruction_name(),
                                engine=inst.engine,
                                sync_info=mybir.SyncInfo(
                                    on_wait=excess[i : i + max_waits], on_update=[]
                                ),
                                bass_nofuse=True,
                            )
                        )
                    inst.sync_info = mybir.SyncInfo(
                        on_wait=keep, on_update=list(si.on_update)
                    )
                    changed = True
                out.append(inst)
            if changed:
                blk.instructions = out


def _plan_blocks(C):
    """Split C into 512-wide blocks plus at most one 256 tail (full-rate
    float32r needs moving dim >= 256). The tail goes last: a narrow first
    block would pull the weight-arrival deadlines into the startup DMA burst."""
    blocks, off = [], 0
    while C - off >= BMAIN:
        blocks.append((off, BMAIN))
        off += BMAIN
    if C - off:
        blocks.append((off, C - off))
    return blocks


def build_nc(C: int):
    """Per-core dense expert MLP: yT = ((gelu(xT.T@w1+b1) @ w2) + b2).T * p."""
    nc = bass.Bass("TRN2", target_bir_lowering=False)
    xT = nc.dram_tensor("xT", (D, C), F32R, kind="ExternalInput")
    w1 = nc.dram_tensor("w1", (D, H), F32R, kind="ExternalInput")
    b1v = nc.dram_tensor("b1v", (P, JH), F32, kind="ExternalInput")
    w2 = nc.dram_tensor("w2", (H, D), F32R, kind="ExternalInput")
    b2v = nc.dram_tensor("b2v", (P, KD), F32, kind="ExternalInput")
    pv = nc.dram_tensor("pv", (P, C), F32, kind="ExternalInput")
    yT = nc.dram_tensor("yT", (D, C), F32, kind="ExternalOutput")

    xT_t = xT.rearrange("(k p) c -> p k c", p=P)
    yT_t = yT.rearrange("(k p) c -> p k c", p=P)

    with tile.TileContext(nc) as tc:
        with (
            tc.tile_pool(name="wpool", bufs=1) as wpool,
            tc.tile_pool(name="xpool", bufs=2) as xpool,
            tc.tile_pool(name="hpool", bufs=1) as hpool,
            tc.tile_pool(name="ypool", bufs=3) as ypool,
            tc.tile_pool(name="psum", bufs=3, space="PSUM") as psum,
        ):
            blocks = _plan_blocks(C)

            KH = KD // 2

            def load_block(off, B):
                # two half-tiles: the first matmul chain waits on 1MB, not 2MB
                xa = xpool.tile([P, KH, B], F32R, tag="xa")
                nc.sync.dma_start(xa[:], xT_t[:, :KH, off : off + B])
                xc = xpool.tile([P, KH, B], F32R, tag="xc")
                nc.sync.dma_start(xc[:], xT_t[:, KH:, off : off + B])
                pb = xpool.tile([P, B], F32, tag="pb")
                nc.sync.dma_start(pb[:], pv[:, off : off + B])
                return (xa, xc), pb

            # Hand-ordered DMA issue: the sync HWDGE queues carry the token
            # stream plus the earliest-needed weight slices (they start fast);
            # the gpsimd SWDGE queues carry the rest of the weights in
            # parallel. Per-output-tile weight slices mean a matmul chain only
            # waits for its own 0.5MB, not the whole 16MB.
            w1_t = w1.rearrange("(k p) h -> p k h", p=P)
            w2_t = w2.rearrange("(j p) d -> p j d", p=P)
            w1sb = [wpool.tile([P, KD, P], F32R, tag=f"w1_{j}", name=f"w1_{j}") for j in range(JH)]
            w2sb = [wpool.tile([P, JH, P], F32R, tag=f"w2_{d}", name=f"w2_{d}") for d in range(KD)]

            def load_w1(j, eng):
                eng.dma_start(w1sb[j][:], w1_t[:, :, j * P : (j + 1) * P])

            def load_w2(d, eng):
                eng.dma_start(w2sb[d][:], w2_t[:, :, d * P : (d + 1) * P])

            # DMA paths: the SWDGE (gpsimd) stream starts ~20us late, so the
            # first four w1 slices ride the sync HWDGE queues interleaved with
            # block 0's token tiles — the PE gets going at ~15us and SWDGE
            # catches up from w1[4] on. Everything else rides SWDGE so the
            # token stream stays unobstructed.
            b1sb = wpool.tile([P, JH], F32)
            b2sb = wpool.tile([P, KD], F32)
            nc.gpsimd.dma_start(b1sb[:], b1v[:])
            nc.gpsimd.dma_start(b2sb[:], b2v[:])
            for j in range(4, JH):
                load_w1(j, nc.gpsimd)
            for d in range(KD):
                load_w2(d, nc.gpsimd)

            load_w1(0, nc.sync)
            off0, B0 = blocks[0]
            xa0 = xpool.tile([P, KH, B0], F32R, tag="xa")
            nc.sync.dma_start(xa0[:], xT_t[:, :KH, off0 : off0 + B0])
            load_w1(1, nc.sync)
            xc0 = xpool.tile([P, KH, B0], F32R, tag="xc")
            nc.sync.dma_start(xc0[:], xT_t[:, KH:, off0 : off0 + B0])
            load_w1(2, nc.sync)
            pb0 = xpool.tile([P, B0], F32, tag="pb")
            nc.sync.dma_start(pb0[:], pv[:, off0 : off0 + B0])
            load_w1(3, nc.sync)

            for bi, (off, B) in enumerate(blocks):
                cs = slice(off, off + B)
                if bi == 0:
                    (xa, xc), pb = (xa0, xc0), pb0
                else:
                    (xa, xc), pb = load_block(off, B)
                hb = hpool.tile([P, JH, B], F32R, tag="hb")
                # h^T[j] = gelu(W1[:, j].T @ x^T + b1[j])
                for j in range(JH):
                    ph = psum.tile([P, B], F32, tag="ph")
                    for k in range(KD):
                        nc.tensor.matmul(
                            ph[:],
                            w1sb[j][:, k],
                            xa[:, k] if k < KH else xc[:, k - KH],
                            start=(k == 0),
                            stop=(k == KD - 1),
                        )
                    nc.scalar.activation(
                        hb[:, j],
                        ph[:],
                        mybir.ActivationFunctionType.Gelu,
                        bias=b1sb[:, j : j + 1],
                    )
                # y^T[d] = (W2[:, d].T @ h^T + b2[d]) * p
                for d in range(KD):
                    pd = psum.tile([P, B], F32, tag="pd")
                    for j in range(JH):
                        nc.tensor.matmul(
                            pd[:],
                            w2sb[d][:, j],
                            hb[:, j],
                            start=(j == 0),
                            stop=(j == JH - 1),
                        )
                    yb = ypool.tile([P, B], F32, tag="yb")
                    nc.scalar.activation(
                        yb[:],
                        pd[:],
                        mybir.ActivationFunctionType.Identity,
                        bias=b2sb[:, d : d + 1],
                    )
                    nc.vector.tensor_mul(yb[:], yb[:], pb[:])
                    nc.sync.dma_start(yT_t[:, d, cs], yb[:])
    _split_excess_waits(nc)
    return nc


_NC_CACHE = {}


def _routing(x, Wg, bg):
    """Gating computed the same way (and on the same platform: CPU jax) as the
    reference, so the top-2 choice is bit-identical even for near-tie logits."""
    import jax
    import jax.numpy as jnp

    cpu = jax.local_devices(backend="cpu")[0]
    with jax.default_device(cpu):
        logits = jnp.asarray(x) @ jnp.asarray(Wg) + jnp.asarray(bg)
        probs = jax.nn.softmax(logits, axis=-1)
        topk_p, topk_i = jax.lax.top_k(probs, TOP_K)
        topk_p = topk_p / topk_p.sum(axis=-1, keepdims=True)
    return np.asarray(topk_i), np.asarray(topk_p)


def kernel(x, Wg, bg, W1, b1, W2, b2):
    global LAST_RESULTS
    x = np.ascontiguousarray(np.asarray(x, dtype=np.float32))
    Wg = np.asarray(Wg, dtype=np.float32)
    bg = np.asarray(bg, dtype=np.float32)
    W1 = np.asarray(W1, dtype=np.float32)
    b1 = np.asarray(b1, dtype=np.float32)
    W2 = np.asarray(W2, dtype=np.float32)
    b2 = np.asarray(b2, dtype=np.float32)

    topk_i, topk_p = _routing(x, Wg, bg)

    idx_list, p_list = [], []
    for e in range(E):
        m0 = topk_i[:, 0] == e
        m1 = topk_i[:, 1] == e
        idx = np.nonzero(m0 | m1)[0]
        p = np.where(m0[idx], topk_p[idx, 0], topk_p[idx, 1]).astype(np.float32)
        idx_list.append(idx)
        p_list.append(p)

    cmax = max(len(i) for i in idx_list)
    C = max(BGRAIN, ((cmax + BGRAIN - 1) // BGRAIN) * BGRAIN)

    if C not in _NC_CACHE:
        _NC_CACHE[C] = build_nc(C)
    nc = _NC_CACHE[C]

    in_maps = []
    for e in range(E):
        idx = idx_list[e]
        n = len(idx)
        xTg = np.zeros((D, C), np.float32)
        xTg[:, :n] = x[idx].T
        pvv = np.zeros((C,), np.float32)
        pvv[:n] = p_list[e]
        pvv = np.ascontiguousarray(np.broadcast_to(pvv, (P, C)))
        in_maps.append(
            {
                "xT": xTg,
                "w1": np.ascontiguousarray(W1[e]),
                "b1v": np.ascontiguousarray(b1[e].reshape(JH, P).T),
                "w2": np.ascontiguousarray(W2[e]),
                "b2v": np.ascontiguousarray(b2[e].reshape(KD, P).T),
                "pv": pvv,
            }
        )

    res = run_bass_kernel_spmd(
        nc, in_maps, core_ids=list(range(E)), trace=TRACE, trace_cores=TRACE_CORES
    )
    LAST_RESULTS = res

    out = x.copy()
    for e in range(E):
        idx = idx_list[e]
        out[idx] += res.results[e]["yT"][:, : len(idx)].T
    return out



# revision 2
# speedup vs baseline: 1.6044x; 1.6044x over previous
"""MoE layer (N=16384, D=1024, E=8, H=2048, top-2) on 8 trn2 NeuronCores.

Strategy: expert parallelism with p-weighted mixed precision. The reference
computes every expert densely but only the top-2 survive the gather — so we
dispatch each token to its two routed experts only (4x compute saving).
Core c owns expert c's weights; the host computes the gating (bit-identically
to the reference, CPU jax) and all-to-all-dispatches gathered token batches.

A routed pair's output error enters the final sum scaled by its routing
weight p, so per expert the tokens are sorted by p: the low-p majority is
computed with float8e4 DoubleRow matmuls (256-wide contraction, 2x PE rate)
and the high-p top-CB tokens with bf16 matmuls (full precision, 1x rate).
Weight scales (x64 on fp8 W1/W2, to clear the e4m3 denormal floor) are folded
into the gelu activation scale and the host-prepared p vector; b2 is folded
into the host-side combine.  The host scatter-adds the two expert
contributions plus the residual.

Self-contained: only numpy/jax/ml_dtypes/concourse imports.
"""
import numpy as np
import ml_dtypes

import concourse.bass as bass
import concourse.mybir as mybir
import concourse.tile as tile
from concourse.bass_utils import run_bass_kernel_spmd

N, D, E, H, TOP_K = 16384, 1024, 8, 2048, 2
P = 128
KD = D // P      # 8  k-tiles over D (bf16 mm1 / output d-tiles)
JH = H // P      # 16 h-tiles over H (bf16 mm2 / gelu tiles)
KP = D // 256    # 4  k-pairs over D (fp8 DoubleRow mm1)
JP = H // 256    # 8  k-pairs over H (fp8 DoubleRow mm2)
BGRAIN = 256     # capacity granularity / tail block width
BMAIN = 512      # main token block (moving dim per matmul)

CB = 1024        # bf16 (high-p) tier capacity per expert, multiple of BGRAIN
WS = 64.0        # fp8 weight scale (keeps quantized weights out of denormals)

TRACE = False          # test harness may flip this
TRACE_CORES = None     # e.g. list(range(8)) to profile every core
LAST_RESULTS = None    # BassKernelResults of the last device run

F32 = mybir.dt.float32
BF16 = mybir.dt.bfloat16
F8 = mybir.dt.float8e4
DR = mybir.MatmulPerfMode.DoubleRow
NP_FP8 = ml_dtypes.float8_e4m3
NP_BF16 = ml_dtypes.bfloat16


def _split_excess_waits(nc, max_waits=1):
    """This walrus build rejects >1 sem-wait per instruction; Tile emits more.
    Move excess waits onto same-engine NOPs inserted right before."""
    for fn in nc.m.functions:
        for blk in fn.blocks:
            insts = list(blk.instructions)
            out = []
            changed = False
            for inst in insts:
                si = getattr(inst, "sync_info", None)
                if si is not None and si.on_wait and len(si.on_wait) > max_waits:
                    waits = list(si.on_wait)
                    excess, keep = waits[:-max_waits], waits[-max_waits:]
                    for i in range(0, len(excess), max_waits):
                        out.append(
                            mybir.InstNoOp(
                                name=nc.get_next_instruction_name(),
                                engine=inst.engine,
                                sync_info=mybir.SyncInfo(
                                    on_wait=excess[i : i + max_waits], on_update=[]
                                ),
                                bass_nofuse=True,
                            )
                        )
                    inst.sync_info = mybir.SyncInfo(
                        on_wait=keep, on_update=list(si.on_update)
                    )
                    changed = True
                out.append(inst)
            if changed:
                blk.instructions = out


def _plan_blocks(C):
    """Split C into 512-wide blocks plus at most one 256 tail (last)."""
    blocks, off = [], 0
    while C - off >= BMAIN:
        blocks.append((off, BMAIN))
        off += BMAIN
    if C - off:
        blocks.append((off, C - off))
    return blocks


def build_nc(C8: int, CBc: int):
    """Per-core dual-tier expert MLP.

    fp8 tier (C8 cols):  yT = (gelu(x8.T@w1q*(1/WS)+b1) @ w2q) .* (p/WS)
    bf16 tier (CBc cols): yT = (gelu(xb.T@w1b+b1) @ w2b) .* p
    """
    nc = bass.Bass("TRN2", target_bir_lowering=False)
    x8 = nc.dram_tensor("x8", (P, KP, 2, C8), F8, kind="ExternalInput")
    xb = nc.dram_tensor("xb", (P, KD, CBc), BF16, kind="ExternalInput")
    w1q = nc.dram_tensor("w1q", (P, KP, 2, H), F8, kind="ExternalInput")
    w2q = nc.dram_tensor("w2q", (P, JP, 2, D), F8, kind="ExternalInput")
    w1b = nc.dram_tensor("w1b", (P, KD, H), BF16, kind="ExternalInput")
    w2b = nc.dram_tensor("w2b", (P, JH, D), BF16, kind="ExternalInput")
    b1v = nc.dram_tensor("b1v", (P, JH), F32, kind="ExternalInput")
    pv8 = nc.dram_tensor("pv8", (P, C8), F32, kind="ExternalInput")
    pvb = nc.dram_tensor("pvb", (P, CBc), F32, kind="ExternalInput")
    y8T = nc.dram_tensor("y8T", (P, KD, C8), F32, kind="ExternalOutput")
    ybT = nc.dram_tensor("ybT", (P, KD, CBc), F32, kind="ExternalOutput")

    blocks8 = _plan_blocks(C8)
    blocksB = _plan_blocks(CBc)

    with tile.TileContext(nc) as tc:
        with (
            tc.tile_pool(name="wpool", bufs=1) as wpool,
            tc.tile_pool(name="x8pool", bufs=2) as x8pool,
            tc.tile_pool(name="xbpool", bufs=2) as xbpool,
            tc.tile_pool(name="h8pool", bufs=2) as h8pool,
            tc.tile_pool(name="hbpool", bufs=2) as hbpool,
            tc.tile_pool(name="ypool", bufs=3) as ypool,
            tc.tile_pool(name="psum", bufs=4, space="PSUM") as psum,
        ):
            # ---- weight tiles (resident) ----
            w1q_sb = wpool.tile([P, KP, 2, H], F8)
            w2q_sb = wpool.tile([P, JP, 2, D], F8)
            w1b_sb = wpool.tile([P, KD, H], BF16)
            w2b_sb = wpool.tile([P, JH, D], BF16)
            b1sb = wpool.tile([P, JH], F32)

            def load_x8(off, B):
                xt = x8pool.tile([P, KP, 2, B], F8, tag="x8")
                nc.sync.dma_start(xt[:], x8[:, :, :, off : off + B])
                pt = x8pool.tile([P, B], F32, tag="p8")
                nc.sync.dma_start(pt[:], pv8[:, off : off + B])
                return xt, pt

            def load_xb(off, B):
                xt = xbpool.tile([P, KD, B], BF16, tag="xb")
                nc.gpsimd.dma_start(xt[:], xb[:, :, off : off + B])
                pt = xbpool.tile([P, B], F32, tag="pb")
                nc.gpsimd.dma_start(pt[:], pvb[:, off : off + B])
                return xt, pt

            # ---- DMA schedule ----
            # sync HWDGE: startup-critical stream — fp8 weights interleaved
            # with the first fp8 token blocks, so the PE starts at ~5us and
            # never starves. gpsimd SWDGE (starts ~20us late): everything the
            # bf16 phase needs, plus b1 — it all arrives long before the fp8
            # phase (~10x longer) finishes.
            nc.sync.dma_start(w1q_sb[:, :, :, : H // 2], w1q[:, :, :, : H // 2])
            x8t0, p8t0 = load_x8(*blocks8[0])
            nc.sync.dma_start(w1q_sb[:, :, :, H // 2 :], w1q[:, :, :, H // 2 :])
            nc.sync.dma_start(w2q_sb[:, : JP // 2], w2q[:, : JP // 2])
            x8t1, p8t1 = (
                load_x8(*blocks8[1]) if len(blocks8) > 1 else (None, None)
            )
            nc.sync.dma_start(w2q_sb[:, JP // 2 :], w2q[:, JP // 2 :])

            nc.gpsimd.dma_start(b1sb[:], b1v[:])
            xbt0, pbt0 = load_xb(*blocksB[0])
            xbt1, pbt1 = (
                load_xb(*blocksB[1]) if len(blocksB) > 1 else (None, None)
            )
            nc.gpsimd.dma_start(w1b_sb[:], w1b[:])
            nc.gpsimd.dma_start(w2b_sb[:], w2b[:])

            # ---- fp8 tier ----
            pend8 = [(x8t0, p8t0), (x8t1, p8t1)]
            for bi, (off, B) in enumerate(blocks8):
                xt, pt = pend8[bi]
                if bi + 2 < len(blocks8):
                    pend8.append(load_x8(*blocks8[bi + 2]))
                h8 = h8pool.tile([P, JP, 2, B], F8, tag="h8")
                for j in range(JH):
                    ph = psum.tile([P, B], F32, tag="ph")
                    for kp in range(KP):
                        nc.tensor.matmul(
                            ph[:],
                            w1q_sb[:, kp, :, j * P : (j + 1) * P],
                            xt[:, kp],
                            start=(kp == 0),
                            stop=(kp == KP - 1),
                            perf_mode=DR,
                        )
                    nc.scalar.activation(
                        h8[:, j // 2, j % 2],
                        ph[:],
                        mybir.ActivationFunctionType.Gelu,
                        bias=b1sb[:, j : j + 1],
                        scale=1.0 / WS,
                    )
                for d in range(KD):
                    pd = psum.tile([P, B], F32, tag="pd")
                    for t in range(JP):
                        nc.tensor.matmul(
                            pd[:],
                            w2q_sb[:, t, :, d * P : (d + 1) * P],
                            h8[:, t],
                            start=(t == 0),
                            stop=(t == JP - 1),
                            perf_mode=DR,
                        )
                    yb = ypool.tile([P, B], F32, tag="yb")
                    nc.vector.tensor_mul(yb[:], pd[:], pt[:])
                    nc.sync.dma_start(y8T[:, d, off : off + B], yb[:])

            # ---- bf16 tier ----
            pendB = [(xbt0, pbt0), (xbt1, pbt1)]
            for bi, (off, B) in enumerate(blocksB):
                xt, pt = pendB[bi]
                if bi + 2 < len(blocksB):
                    pendB.append(load_xb(*blocksB[bi + 2]))
                hb = hbpool.tile([P, JH, B], BF16, tag="hb")
                for j in range(JH):
                    ph = psum.tile([P, B], F32, tag="ph")
                    for k in range(KD):
                        nc.tensor.matmul(
                            ph[:],
                            w1b_sb[:, k, j * P : (j + 1) * P],
                            xt[:, k],
                            start=(k == 0),
                            stop=(k == KD - 1),
                        )
                    nc.scalar.activation(
                        hb[:, j],
                        ph[:],
                        mybir.ActivationFunctionType.Gelu,
                        bias=b1sb[:, j : j + 1],
                    )
                for d in range(KD):
                    pd = psum.tile([P, B], F32, tag="pd")
                    for j in range(JH):
                        nc.tensor.matmul(
                            pd[:],
                            w2b_sb[:, j, d * P : (d + 1) * P],
                            hb[:, j],
                            start=(j == 0),
                            stop=(j == JH - 1),
                        )
                    yb = ypool.tile([P, B], F32, tag="yb")
                    nc.vector.tensor_mul(yb[:], pd[:], pt[:])
                    nc.sync.dma_start(ybT[:, d, off : off + B], yb[:])
    _split_excess_waits(nc)
    return nc


_NC_CACHE = {}


def _routing(x, Wg, bg):
    """Gating computed the same way (and on the same platform: CPU jax) as the
    reference, so the top-2 choice is bit-identical even for near-tie logits."""
    import jax
    import jax.numpy as jnp

    cpu = jax.local_devices(backend="cpu")[0]
    with jax.default_device(cpu):
        logits = jnp.asarray(x) @ jnp.asarray(Wg) + jnp.asarray(bg)
        probs = jax.nn.softmax(logits, axis=-1)
        topk_p, topk_i = jax.lax.top_k(probs, TOP_K)
        topk_p = topk_p / topk_p.sum(axis=-1, keepdims=True)
    return np.asarray(topk_i), np.asarray(topk_p)


def _pack_fp8_dr(w, scale):
    """[K, M] f32 -> [P, K//256, 2, M] fp8 DoubleRow stationary layout."""
    K, M = w.shape
    q = (w * scale).astype(NP_FP8)
    return np.ascontiguousarray(q.reshape(K // 256, 2, P, M).transpose(2, 0, 1, 3))


def kernel(x, Wg, bg, W1, b1, W2, b2):
    global LAST_RESULTS
    x = np.ascontiguousarray(np.asarray(x, dtype=np.float32))
    Wg = np.asarray(Wg, dtype=np.float32)
    bg = np.asarray(bg, dtype=np.float32)
    W1 = np.asarray(W1, dtype=np.float32)
    b1 = np.asarray(b1, dtype=np.float32)
    W2 = np.asarray(W2, dtype=np.float32)
    b2 = np.asarray(b2, dtype=np.float32)

    topk_i, topk_p = _routing(x, Wg, bg)

    # Per expert: tokens sorted by routing weight ascending; top-CB run bf16.
    idx8_l, p8_l, idxB_l, pB_l = [], [], [], []
    for e in range(E):
        m0 = topk_i[:, 0] == e
        m1 = topk_i[:, 1] == e
        idx = np.nonzero(m0 | m1)[0]
        pe = np.where(m0[idx], topk_p[idx, 0], topk_p[idx, 1]).astype(np.float32)
        order = np.argsort(pe, kind="stable")
        idx, pe = idx[order], pe[order]
        nB = min(CB, len(idx))
        n8 = len(idx) - nB
        idx8_l.append(idx[:n8])
        p8_l.append(pe[:n8])
        idxB_l.append(idx[n8:])
        pB_l.append(pe[n8:])

    c8max = max(len(i) for i in idx8_l)
    C8 = max(BGRAIN, ((c8max + BGRAIN - 1) // BGRAIN) * BGRAIN)

    key = (C8, CB)
    if key not in _NC_CACHE:
        _NC_CACHE[key] = build_nc(C8, CB)
    nc = _NC_CACHE[key]

    in_maps = []
    for e in range(E):
        i8, p8v = idx8_l[e], p8_l[e]
        iB, pBv = idxB_l[e], pB_l[e]
        n8, nB = len(i8), len(iB)

        x8 = np.zeros((P, KP, 2, C8), NP_FP8)
        if n8:
            xq = x[i8].T.astype(NP_FP8)  # [D, n8]
            x8[:, :, :, :n8] = xq.reshape(KP, 2, P, n8).transpose(2, 0, 1, 3)
        xbb = np.zeros((P, KD, CB), NP_BF16)
        if nB:
            xqb = x[iB].T.astype(NP_BF16)  # [D, nB]
            xbb[:, :, :nB] = xqb.reshape(KD, P, nB).transpose(1, 0, 2)

        pv8 = np.zeros((C8,), np.float32)
        pv8[:n8] = p8v / WS
        pvb = np.zeros((CB,), np.float32)
        pvb[:nB] = pBv

        in_maps.append(
            {
                "x8": x8,
                "xb": xbb,
                "w1q": _pack_fp8_dr(W1[e], WS),
                "w2q": _pack_fp8_dr(W2[e], WS),
                "w1b": np.ascontiguousarray(
                    W1[e].astype(NP_BF16).reshape(KD, P, H).transpose(1, 0, 2)
                ),
                "w2b": np.ascontiguousarray(
                    W2[e].astype(NP_BF16).reshape(JH, P, D).transpose(1, 0, 2)
                ),
                "b1v": np.ascontiguousarray(b1[e].reshape(JH, P).T),
                "pv8": np.ascontiguousarray(np.broadcast_to(pv8, (P, C8))),
                "pvb": np.ascontiguousarray(np.broadcast_to(pvb, (P, CB))),
            }
        )

    res = run_bass_kernel_spmd(
        nc, in_maps, core_ids=list(range(E)), trace=TRACE, trace_cores=TRACE_CORES
    )
    LAST_RESULTS = res

    out = x.copy()
    if np.any(b2):
        out += topk_p[:, 0:1] * b2[topk_i[:, 0]] + topk_p[:, 1:2] * b2[topk_i[:, 1]]
    for e in range(E):
        i8, iB = idx8_l[e], idxB_l[e]
        if len(i8):
            y8 = res.results[e]["y8T"][:, :, : len(i8)]  # [P, KD, n8]
            out[i8] += y8.transpose(2, 1, 0).reshape(len(i8), D)
        if len(iB):
            yB = res.results[e]["ybT"][:, :, : len(iB)]
            out[iB] += yB.transpose(2, 1, 0).reshape(len(iB), D)
    return out


# revision 3
# speedup vs baseline: 1.8070x; 1.1263x over previous
"""MoE layer (N=16384, D=1024, E=8, H=2048, top-2) on 8 trn2 NeuronCores.

Strategy: expert parallelism with p-weighted mixed precision. The reference
computes every expert densely but only the top-2 survive the gather — so we
dispatch each token to its two routed experts only (4x compute saving).
Core c owns expert c's weights; the host computes the gating (bit-identically
to the reference, CPU jax) and all-to-all-dispatches gathered token batches.

A routed pair's output error enters the final sum scaled by its routing
weight p, so per expert the tokens are sorted by p: the low-p majority is
computed with float8e4 DoubleRow matmuls (256-wide contraction, 2x PE rate)
and the high-p top-CB tokens with bf16 matmuls (full precision, 1x rate).
Weight scales (x64 on fp8 W1/W2, to clear the e4m3 denormal floor) are folded
into the gelu activation scale and the host-prepared p vector; b2 is folded
into the host-side combine. The host scatter-adds the two expert
contributions plus the residual.

DMA plan (from v1 trace analysis: HWDGE queues start ~8.5us and run
~220GB/s each, but an early SWDGE burst starves them to ~13GB/s): the two
HWDGE queues (sync + scalar) split the startup-critical fp8 weights and the
token stream; the bulky bf16 weights and bf16 tokens are paced through the
scalar queue between successive blocks' activations; SWDGE carries only b1.

Self-contained: only numpy/jax/ml_dtypes/concourse imports.
"""
import numpy as np
import ml_dtypes

import concourse.bass as bass
import concourse.mybir as mybir
import concourse.tile as tile
from concourse.bass_utils import run_bass_kernel_spmd

N, D, E, H, TOP_K = 16384, 1024, 8, 2048, 2
P = 128
KD = D // P      # 8  k-tiles over D (bf16 mm1 / output d-tiles)
JH = H // P      # 16 h-tiles over H (bf16 mm2 / gelu tiles)
KP = D // 256    # 4  k-pairs over D (fp8 DoubleRow mm1)
JP = H // 256    # 8  k-pairs over H (fp8 DoubleRow mm2)
BGRAIN = 256     # capacity granularity / tail block width
BMAIN = 512      # main token block (moving dim per matmul)

CB = 768         # bf16 (high-p) tier capacity per expert, multiple of BGRAIN
WS = 64.0        # fp8 weight scale (keeps quantized weights out of denormals)

TRACE = False          # test harness may flip this
TRACE_CORES = None     # e.g. list(range(8)) to profile every core
LAST_RESULTS = None    # BassKernelResults of the last device run

F32 = mybir.dt.float32
BF16 = mybir.dt.bfloat16
F8 = mybir.dt.float8e4
DR = mybir.MatmulPerfMode.DoubleRow
NP_FP8 = ml_dtypes.float8_e4m3
NP_BF16 = ml_dtypes.bfloat16


def _split_excess_waits(nc, max_waits=1):
    """This walrus build rejects >1 sem-wait per instruction; Tile emits more.
    Move excess waits onto same-engine NOPs inserted right before."""
    for fn in nc.m.functions:
        for blk in fn.blocks:
            insts = list(blk.instructions)
            out = []
            changed = False
            for inst in insts:
                si = getattr(inst, "sync_info", None)
                if si is not None and si.on_wait and len(si.on_wait) > max_waits:
                    waits = list(si.on_wait)
                    excess, keep = waits[:-max_waits], waits[-max_waits:]
                    for i in range(0, len(excess), max_waits):
                        out.append(
                            mybir.InstNoOp(
                                name=nc.get_next_instruction_name(),
                                engine=inst.engine,
                                sync_info=mybir.SyncInfo(
                                    on_wait=excess[i : i + max_waits], on_update=[]
                                ),
                                bass_nofuse=True,
                            )
                        )
                    inst.sync_info = mybir.SyncInfo(
                        on_wait=keep, on_update=list(si.on_update)
                    )
                    changed = True
                out.append(inst)
            if changed:
                blk.instructions = out


def _plan_blocks(C):
    """Split C into 512-wide blocks plus at most one 256 tail (last)."""
    blocks, off = [], 0
    while C - off >= BMAIN:
        blocks.append((off, BMAIN))
        off += BMAIN
    if C - off:
        blocks.append((off, C - off))
    return blocks


def build_nc(C8: int, CBc: int):
    """Per-core dual-tier expert MLP.

    fp8 tier (C8 cols):  yT = (gelu(x8.T@w1q*(1/WS)+b1) @ w2q) .* (p/WS)
    bf16 tier (CBc cols): yT = (gelu(xb.T@w1b+b1) @ w2b) .* p
    """
    nc = bass.Bass("TRN2", target_bir_lowering=False)
    x8 = nc.dram_tensor("x8", (P, KP, 2, C8), F8, kind="ExternalInput")
    xb = nc.dram_tensor("xb", (P, KD, CBc), BF16, kind="ExternalInput")
    w1q = nc.dram_tensor("w1q", (P, KP, 2, H), F8, kind="ExternalInput")
    w2q = nc.dram_tensor("w2q", (P, JP, 2, D), F8, kind="ExternalInput")
    w1b = nc.dram_tensor("w1b", (P, KD, H), BF16, kind="ExternalInput")
    w2b = nc.dram_tensor("w2b", (P, JH, D), BF16, kind="ExternalInput")
    b1v = nc.dram_tensor("b1v", (P, JH), F32, kind="ExternalInput")
    pv8 = nc.dram_tensor("pv8", (P, C8), F32, kind="ExternalInput")
    pvb = nc.dram_tensor("pvb", (P, CBc), F32, kind="ExternalInput")
    y8T = nc.dram_tensor("y8T", (P, KD, C8), F32, kind="ExternalOutput")
    ybT = nc.dram_tensor("ybT", (P, KD, CBc), F32, kind="ExternalOutput")

    blocks8 = _plan_blocks(C8)
    blocksB = _plan_blocks(CBc)
    nb8 = len(blocks8)

    with tile.TileContext(nc) as tc:
        with (
            tc.tile_pool(name="wpool", bufs=1) as wpool,
            tc.tile_pool(name="x8pool", bufs=2) as x8pool,
            tc.tile_pool(name="xbpool", bufs=2) as xbpool,
            tc.tile_pool(name="h8pool", bufs=2) as h8pool,
            tc.tile_pool(name="hbpool", bufs=2) as hbpool,
            tc.tile_pool(name="ypool", bufs=6) as ypool,
            tc.tile_pool(name="psum", bufs=4, space="PSUM") as psum,
        ):
            # ---- weight tiles (resident) ----
            w1q_sb = wpool.tile([P, KP, 2, H], F8)
            w2q_sb = wpool.tile([P, JP, 2, D], F8)
            w1b_sb = wpool.tile([P, KD, H], BF16)
            w2b_sb = wpool.tile([P, JH, D], BF16)
            b1sb = wpool.tile([P, JH], F32)

            # All per-block tiles are allocated 512 wide and sliced to B so
            # every pool tag has exactly one shape (256 tails share buffers).
            def load_x8(off, B):
                xt = x8pool.tile([P, KP, 2, BMAIN], F8, tag="x8")
                nc.sync.dma_start(xt[:, :, :, :B], x8[:, :, :, off : off + B])
                pt = x8pool.tile([P, BMAIN], F32, tag="p8")
                nc.sync.dma_start(pt[:, :B], pv8[:, off : off + B])
                return xt, pt

            def load_xb(off, B):
                xt = xbpool.tile([P, KD, BMAIN], BF16, tag="xb")
                nc.scalar.dma_start(xt[:, :, :B], xb[:, :, off : off + B])
                pt = xbpool.tile([P, BMAIN], F32, tag="pb")
                nc.scalar.dma_start(pt[:, :B], pvb[:, off : off + B])
                return xt, pt

            # ---- startup DMAs ----
            # sync: x8 block0 first (smallest critical prefix), then fp8
            # weight chunks interleaved with the next token blocks.
            # scalar: the other half of the fp8 weights, concurrently.
            H4 = H // 4
            x8t0, p8t0 = load_x8(*blocks8[0])
            nc.sync.dma_start(w1q_sb[:, :, :, :H4], w1q[:, :, :, :H4])
            nc.scalar.dma_start(w1q_sb[:, :, :, H4 : 2 * H4], w1q[:, :, :, H4 : 2 * H4])
            nc.sync.dma_start(
                w1q_sb[:, :, :, 2 * H4 : 3 * H4], w1q[:, :, :, 2 * H4 : 3 * H4]
            )
            nc.scalar.dma_start(w1q_sb[:, :, :, 3 * H4 :], w1q[:, :, :, 3 * H4 :])
            nc.sync.dma_start(w2q_sb[:, :, :, : D // 2], w2q[:, :, :, : D // 2])
            nc.scalar.dma_start(w2q_sb[:, :, :, D // 2 :], w2q[:, :, :, D // 2 :])
            x8t1, p8t1 = load_x8(*blocks8[1]) if nb8 > 1 else (None, None)
            nc.gpsimd.dma_start(b1sb[:], b1v[:])

            # Bulky bf16-tier loads, paced through the scalar queue after
            # block k's gelus (k: chunk) so they never contend with the
            # startup-critical stream. Everything lands long before the bf16
            # phase starts.
            pendB = []

            def paced_loads(k):
                H2, D2 = H // 2, D // 2
                if k == 0:
                    nc.scalar.dma_start(w1b_sb[:, :, :H2], w1b[:, :, :H2])
                elif k == 1:
                    nc.scalar.dma_start(w1b_sb[:, :, H2:], w1b[:, :, H2:])
                elif k == 2:
                    nc.scalar.dma_start(w2b_sb[:, :, :D2], w2b[:, :, :D2])
                elif k == 3:
                    nc.scalar.dma_start(w2b_sb[:, :, D2:], w2b[:, :, D2:])
                elif k == 4:
                    if len(blocksB) > 0:
                        pendB.append(load_xb(*blocksB[0]))
                elif k == 5:
                    if len(blocksB) > 1:
                        pendB.append(load_xb(*blocksB[1]))

            # ---- fp8 tier ----
            pend8 = [(x8t0, p8t0), (x8t1, p8t1)]
            for bi, (off, B) in enumerate(blocks8):
                xt, pt = pend8[bi]
                if bi + 2 < nb8:
                    pend8.append(load_x8(*blocks8[bi + 2]))
                h8 = h8pool.tile([P, JP, 2, BMAIN], F8, tag="h8")
                for j in range(JH):
                    ph = psum.tile([P, BMAIN], F32, tag="ph")
                    for kp in range(KP):
                        nc.tensor.matmul(
                            ph[:, :B],
                            w1q_sb[:, kp, :, j * P : (j + 1) * P],
                            xt[:, kp, :, :B],
                            start=(kp == 0),
                            stop=(kp == KP - 1),
                            perf_mode=DR,
                        )
                    nc.scalar.activation(
                        h8[:, j // 2, j % 2, :B],
                        ph[:, :B],
                        mybir.ActivationFunctionType.Gelu,
                        bias=b1sb[:, j : j + 1],
                        scale=1.0 / WS,
                    )
                paced_loads(bi)
                for d in range(KD):
                    pd = psum.tile([P, BMAIN], F32, tag="pd")
                    for t in range(JP):
                        nc.tensor.matmul(
                            pd[:, :B],
                            w2q_sb[:, t, :, d * P : (d + 1) * P],
                            h8[:, t, :, :B],
                            start=(t == 0),
                            stop=(t == JP - 1),
                            perf_mode=DR,
                        )
                    yb = ypool.tile([P, BMAIN], F32, tag="yb")
                    nc.vector.tensor_mul(yb[:, :B], pd[:, :B], pt[:, :B])
                    nc.sync.dma_start(y8T[:, d, off : off + B], yb[:, :B])
            for k in range(nb8, 6):
                paced_loads(k)

            # ---- bf16 tier ----
            for bi, (off, B) in enumerate(blocksB):
                xt, pt = pendB[bi]
                last = bi == len(blocksB) - 1
                hb = hbpool.tile([P, JH, BMAIN], BF16, tag="hb")
                for j in range(JH):
                    ph = psum.tile([P, BMAIN], F32, tag="ph")
                    for k in range(KD):
                        nc.tensor.matmul(
                            ph[:, :B],
                            w1b_sb[:, k, j * P : (j + 1) * P],
                            xt[:, k, :B],
                            start=(k == 0),
                            stop=(k == KD - 1),
                        )
                    nc.scalar.activation(
                        hb[:, j, :B],
                        ph[:, :B],
                        mybir.ActivationFunctionType.Gelu,
                        bias=b1sb[:, j : j + 1],
                    )
                for d in range(KD):
                    pd = psum.tile([P, BMAIN], F32, tag="pd")
                    for j in range(JH):
                        nc.tensor.matmul(
                            pd[:, :B],
                            w2b_sb[:, j, d * P : (d + 1) * P],
                            hb[:, j, :B],
                            start=(j == 0),
                            stop=(j == JH - 1),
                        )
                    yb = ypool.tile([P, BMAIN], F32, tag="yb")
                    nc.vector.tensor_mul(yb[:, :B], pd[:, :B], pt[:, :B])
                    # split the final block's output burst across both HWDGE
                    # queues to shorten the tail drain
                    eng = nc.scalar if (last and d % 2) else nc.sync
                    eng.dma_start(ybT[:, d, off : off + B], yb[:, :B])
    _split_excess_waits(nc)
    return nc


_NC_CACHE = {}


def _routing(x, Wg, bg):
    """Gating computed the same way (and on the same platform: CPU jax) as the
    reference, so the top-2 choice is bit-identical even for near-tie logits."""
    import jax
    import jax.numpy as jnp

    cpu = jax.local_devices(backend="cpu")[0]
    with jax.default_device(cpu):
        logits = jnp.asarray(x) @ jnp.asarray(Wg) + jnp.asarray(bg)
        probs = jax.nn.softmax(logits, axis=-1)
        topk_p, topk_i = jax.lax.top_k(probs, TOP_K)
        topk_p = topk_p / topk_p.sum(axis=-1, keepdims=True)
    return np.asarray(topk_i), np.asarray(topk_p)


def _pack_fp8_dr(w, scale):
    """[K, M] f32 -> [P, K//256, 2, M] fp8 DoubleRow stationary layout."""
    K, M = w.shape
    q = (w * scale).astype(NP_FP8)
    return np.ascontiguousarray(q.reshape(K // 256, 2, P, M).transpose(2, 0, 1, 3))


def kernel(x, Wg, bg, W1, b1, W2, b2):
    global LAST_RESULTS
    x = np.ascontiguousarray(np.asarray(x, dtype=np.float32))
    Wg = np.asarray(Wg, dtype=np.float32)
    bg = np.asarray(bg, dtype=np.float32)
    W1 = np.asarray(W1, dtype=np.float32)
    b1 = np.asarray(b1, dtype=np.float32)
    W2 = np.asarray(W2, dtype=np.float32)
    b2 = np.asarray(b2, dtype=np.float32)

    topk_i, topk_p = _routing(x, Wg, bg)

    # Per expert: tokens sorted by routing weight ascending; top-CB run bf16.
    idx8_l, p8_l, idxB_l, pB_l = [], [], [], []
    for e in range(E):
        m0 = topk_i[:, 0] == e
        m1 = topk_i[:, 1] == e
        idx = np.nonzero(m0 | m1)[0]
        pe = np.where(m0[idx], topk_p[idx, 0], topk_p[idx, 1]).astype(np.float32)
        order = np.argsort(pe, kind="stable")
        idx, pe = idx[order], pe[order]
        nB = min(CB, len(idx))
        n8 = len(idx) - nB
        idx8_l.append(idx[:n8])
        p8_l.append(pe[:n8])
        idxB_l.append(idx[n8:])
        pB_l.append(pe[n8:])

    c8max = max(len(i) for i in idx8_l)
    C8 = max(BGRAIN, ((c8max + BGRAIN - 1) // BGRAIN) * BGRAIN)

    key = (C8, CB)
    if key not in _NC_CACHE:
        _NC_CACHE[key] = build_nc(C8, CB)
    nc = _NC_CACHE[key]

    in_maps = []
    for e in range(E):
        i8, p8v = idx8_l[e], p8_l[e]
        iB, pBv = idxB_l[e], pB_l[e]
        n8, nB = len(i8), len(iB)

        x8 = np.zeros((P, KP, 2, C8), NP_FP8)
        if n8:
            xq = x[i8].T.astype(NP_FP8)  # [D, n8]
            x8[:, :, :, :n8] = xq.reshape(KP, 2, P, n8).transpose(2, 0, 1, 3)
        xbb = np.zeros((P, KD, CB), NP_BF16)
        if nB:
            xqb = x[iB].T.astype(NP_BF16)  # [D, nB]
            xbb[:, :, :nB] = xqb.reshape(KD, P, nB).transpose(1, 0, 2)

        pv8 = np.zeros((C8,), np.float32)
        pv8[:n8] = p8v / WS
        pvb = np.zeros((CB,), np.float32)
        pvb[:nB] = pBv

        in_maps.append(
            {
                "x8": x8,
                "xb": xbb,
                "w1q": _pack_fp8_dr(W1[e], WS),
                "w2q": _pack_fp8_dr(W2[e], WS),
                "w1b": np.ascontiguousarray(
                    W1[e].astype(NP_BF16).reshape(KD, P, H).transpose(1, 0, 2)
                ),
                "w2b": np.ascontiguousarray(
                    W2[e].astype(NP_BF16).reshape(JH, P, D).transpose(1, 0, 2)
                ),
                "b1v": np.ascontiguousarray(b1[e].reshape(JH, P).T),
                "pv8": np.ascontiguousarray(np.broadcast_to(pv8, (P, C8))),
                "pvb": np.ascontiguousarray(np.broadcast_to(pvb, (P, CB))),
            }
        )

    res = run_bass_kernel_spmd(
        nc, in_maps, core_ids=list(range(E)), trace=TRACE, trace_cores=TRACE_CORES
    )
    LAST_RESULTS = res

    out = x.copy()
    if np.any(b2):
        out += topk_p[:, 0:1] * b2[topk_i[:, 0]] + topk_p[:, 1:2] * b2[topk_i[:, 1]]
    for e in range(E):
        i8, iB = idx8_l[e], idxB_l[e]
        if len(i8):
            y8 = res.results[e]["y8T"][:, :, : len(i8)]  # [P, KD, n8]
            out[i8] += y8.transpose(2, 1, 0).reshape(len(i8), D)
        if len(iB):
            yB = res.results[e]["ybT"][:, :, : len(iB)]
            out[iB] += yB.transpose(2, 1, 0).reshape(len(iB), D)
    return out
